# revision 4
# baseline (speedup 1.0000x reference)
"""MoE depthwise-expert routing kernel for 8 Trainium2 NeuronCores.

Strategy (hardcoded for B=32, C=64, H=W=192, E=6, K=3):
  - Data-parallel over batch: 4 samples/core, processed as 2 slabs of
    (2 samples x 64 channels) = 128 SBUF partitions; spatial dims in the
    free dimension, so conv taps are free-dim offsets and per-channel
    weights are diagonal matmul weights.
  - Launch 1 (device): max+sum pooling over HxW per (sample, channel).
  - Host glue: tiny gate MLP + top-k + softmax (12K FLOPs) selects 3
    experts/sample; packs block-diagonal bf16 weight matrices with the
    softmax coefficient folded into conv2 weights/biases.
  - Launch 2 (device): both depthwise 3x3 convs for the 3 selected
    experts as diagonal matmuls on the TensorE accumulating in PSUM
    (9 taps conv1 -> ACT relu+bias evict; 27 taps conv2 -> ACT bias
    evict) in H-strips with a width-padded layout.
"""

import sys

sys.path.insert(0, "/opt/trn_rl_repo")

import numpy as np
import ml_dtypes

from concourse import bass, bacc, tile
from concourse import mybir
from concourse.bass_utils import run_bass_kernel_spmd

F32 = mybir.dt.float32
BF16 = mybir.dt.bfloat16
AF = mybir.ActivationFunctionType

B, C, H, W = 32, 64, 192, 192
E, TOPK = 6, 3
NCORES = 8
SLABS = 2            # slabs per core, each 2 samples x 64 ch = 128 partitions
HWF = H * W          # 36864 free elems per (sample, channel)
WP = W + 2           # padded row width
R = 32               # output rows per strip
NSTRIP = H // R      # 6
XROWS = R + 4        # x rows per strip (2 halo each side)
LEAD = 2             # flat-buffer lead/trail pad elems
XFLAT = LEAD + XROWS * WP + LEAD
YROWS = R + 4        # y buffer rows: row 0 & 35 scratch, rows 1..34 = y halo+body
YFLAT = LEAD + YROWS * WP + LEAD
NC1 = (R + 2) // 2   # conv1 2-row chunks (17)
NC2 = R // 2         # conv2 2-row chunks (16)
TAPS = [(dy, dx) for dy in (-1, 0, 1) for dx in (-1, 0, 1)]

_cache = {}
_last_inmaps = {}


def _build_pool_program():
    nc = bacc.Bacc(None, target_bir_lowering=False, debug=False)
    x = nc.dram_tensor("x", [SLABS, 128, HWF], F32, kind="ExternalInput")
    pooled = nc.dram_tensor("pooled", [SLABS, 128, 2], F32, kind="ExternalOutput")
    CH = 4608
    NCH = HWF // CH  # 8
    with tile.TileContext(nc) as tc:
        with (
            tc.tile_pool(name="xin", bufs=3) as xpool,
            tc.tile_pool(name="acc", bufs=1) as apool,
            tc.tile_pool(name="res", bufs=2) as rpool,
        ):
            mx = apool.tile([128, SLABS * NCH], F32)
            sm = apool.tile([128, SLABS * NCH], F32)
            for s in range(SLABS):
                for i in range(NCH):
                    xt = xpool.tile([128, CH], F32)
                    nc.sync.dma_start(out=xt[:], in_=x[s, :, i * CH:(i + 1) * CH])
                    col = s * NCH + i
                    nc.vector.tensor_reduce(
                        mx[:, col:col + 1], xt[:],
                        mybir.AxisListType.X, mybir.AluOpType.max)
                    nc.vector.tensor_reduce(
                        sm[:, col:col + 1], xt[:],
                        mybir.AxisListType.X, mybir.AluOpType.add)
            for s in range(SLABS):
                pt = rpool.tile([128, 2], F32)
                nc.vector.tensor_reduce(
                    pt[:, 0:1], mx[:, s * NCH:(s + 1) * NCH],
                    mybir.AxisListType.X, mybir.AluOpType.max)
                nc.vector.tensor_reduce(
                    pt[:, 1:2], sm[:, s * NCH:(s + 1) * NCH],
                    mybir.AxisListType.X, mybir.AluOpType.add)
                nc.sync.dma_start(out=pooled[s], in_=pt[:])
    nc.compile()
    return nc


def _build_conv_program():
    nc = bacc.Bacc(None, target_bir_lowering=False, debug=False)
    xb = nc.dram_tensor("xb", [SLABS, 128, HWF], BF16, kind="ExternalInput")
    wd1 = nc.dram_tensor("wd1", [128, SLABS, TOPK, 9, 128], BF16, kind="ExternalInput")
    wd2 = nc.dram_tensor("wd2", [128, SLABS, TOPK, 9, 128], BF16, kind="ExternalInput")
    b1e = nc.dram_tensor("b1e", [128, SLABS, TOPK], F32, kind="ExternalInput")
    bout = nc.dram_tensor("bout", [128, SLABS], F32, kind="ExternalInput")
    out = nc.dram_tensor("out", [SLABS, 128, HWF], F32, kind="ExternalOutput")

    with tile.TileContext(nc) as tc:
        with (
            tc.tile_pool(name="wts", bufs=1) as wpool,
            tc.tile_pool(name="xs", bufs=2) as xpool,
            tc.tile_pool(name="ys", bufs=2) as ypool,
            tc.tile_pool(name="os", bufs=2) as opool,
            tc.tile_pool(name="ps", bufs=8, space="PSUM") as ppool,
        ):
            w1sb = wpool.tile([128, SLABS, TOPK, 9, 128], BF16)
            w2sb = wpool.tile([128, SLABS, TOPK, 9, 128], BF16)
            b1sb = wpool.tile([128, SLABS, TOPK], F32)
            bosb = wpool.tile([128, SLABS], F32)
            nc.sync.dma_start(out=w1sb[:], in_=wd1[:])
            nc.sync.dma_start(out=w2sb[:], in_=wd2[:])
            nc.sync.dma_start(out=b1sb[:], in_=b1e[:])
            nc.sync.dma_start(out=bosb[:], in_=bout[:])

            for s in range(SLABS):
                for st in range(NSTRIP):
                    h0 = st * R
                    xt = xpool.tile([128, XFLAT], BF16, tag="x")
                    x3 = xt[:, LEAD:LEAD + XROWS * WP].rearrange(
                        "p (r c) -> p r c", c=WP)
                    # zero pad columns (image col -1 and 192)
                    nc.vector.memset(x3[:, :, 0:1], 0.0)
                    nc.vector.memset(x3[:, :, W + 1:W + 2], 0.0)
                    # load x rows [h0-2, h0+R+2) ; memset out-of-image rows
                    if st == 0:
                        nc.vector.memset(x3[:, 0:2, 1:W + 1], 0.0)
                        src = xb[s, :, 0:(R + 2) * W].rearrange(
                            "p (r c) -> p r c", c=W)
                        nc.sync.dma_start(out=x3[:, 2:XROWS, 1:W + 1], in_=src)
                    elif st == NSTRIP - 1:
                        nc.vector.memset(x3[:, XROWS - 2:XROWS, 1:W + 1], 0.0)
                        src = xb[s, :, (h0 - 2) * W:H * W].rearrange(
                            "p (r c) -> p r c", c=W)
                        nc.sync.dma_start(out=x3[:, 0:XROWS - 2, 1:W + 1], in_=src)
                    else:
                        src = xb[s, :, (h0 - 2) * W:(h0 + R + 2) * W].rearrange(
                            "p (r c) -> p r c", c=W)
                        nc.sync.dma_start(out=x3[:, :, 1:W + 1], in_=src)

                    # conv1 + relu for the 3 selected experts
                    yts = []
                    for j in range(TOPK):
                        yt = ypool.tile([128, YFLAT], BF16, tag=f"y{j}")
                        y3 = yt[:, LEAD:LEAD + YROWS * WP].rearrange(
                            "p (r c) -> p r c", c=WP)
                        nc.vector.memset(y3[:, :, 0:1], 0.0)
                        nc.vector.memset(y3[:, :, W + 1:W + 2], 0.0)
                        if st == 0:
                            nc.vector.memset(y3[:, 1:2, 1:W + 1], 0.0)
                        if st == NSTRIP - 1:
                            nc.vector.memset(y3[:, YROWS - 2:YROWS - 1, 1:W + 1], 0.0)
                        yts.append(yt)
                        for cy in range(NC1):
                            ps = ppool.tile([128, 2, WP], F32, tag="ps")
                            for t, (dy, dx) in enumerate(TAPS):
                                off = LEAD + (1 + 2 * cy + dy) * WP + dx
                                nc.tensor.matmul(
                                    ps[:], w1sb[:, s, j, t, :],
                                    xt[:, off:off + 2 * WP],
                                    start=(t == 0), stop=(t == 8))
                            # evict with relu + bias; skip out-of-image rows
                            r_lo, p_lo = 1 + 2 * cy, 0
                            r_hi = r_lo + 2
                            if st == 0 and cy == 0:
                                r_lo, p_lo = 2, 1
                            if st == NSTRIP - 1 and cy == NC1 - 1:
                                r_hi -= 1
                            nr = r_hi - r_lo
                            nc.scalar.activation(
                                y3[:, r_lo:r_hi, 1:W + 1],
                                ps[:, p_lo:p_lo + nr, 1:W + 1],
                                AF.Relu, bias=b1sb[:, s, j:j + 1], scale=1.0)

                    # conv2 over 3 experts, combined into one PSUM accumulation
                    ot = opool.tile([128, R, W], F32, tag="o")
                    for co in range(NC2):
                        ps = ppool.tile([128, 2, WP], F32, tag="ps")
                        k = 0
                        for j in range(TOPK):
                            for t, (dy, dx) in enumerate(TAPS):
                                off = LEAD + (2 * co + 2 + dy) * WP + dx
                                nc.tensor.matmul(
                                    ps[:], w2sb[:, s, j, t, :],
                                    yts[j][:, off:off + 2 * WP],
                                    start=(k == 0), stop=(k == 26))
                                k += 1
                        nc.scalar.activation(
                            ot[:, 2 * co:2 * co + 2, :],
                            ps[:, :, 1:W + 1],
                            AF.Identity, bias=bosb[:, s:s + 1], scale=1.0)
                    nc.sync.dma_start(
                        out=out[s, :, h0 * W:(h0 + R) * W],
                        in_=ot[:].rearrange("p r c -> p (r c)"))
    nc.compile()
    return nc


def _gate_host(pooled, fc0_w, fc0_b, fc1_w, fc1_b):
    """Replicates reference._gate from pooled [B, C] stats; float32."""
    pooled = pooled.astype(np.float32)
    g_lin = pooled @ fc1_w.T + fc1_b
    g = np.where(g_lin > 0, g_lin, 0.2 * g_lin).astype(np.float32)
    n_lin = (pooled @ fc0_w.T + fc0_b).astype(np.float32)
    noise = (np.log1p(np.exp(-np.abs(n_lin))) + np.maximum(n_lin, 0.0)).astype(np.float32)
    mu = noise.mean(axis=1, keepdims=True)
    sd = noise.std(axis=1, ddof=1, keepdims=True)
    nz = (noise - mu) / sd
    scores = g + nz
    idx = np.argsort(-scores, axis=1, kind="stable")[:, :TOPK]
    rows = np.arange(scores.shape[0])[:, None]
    mask = np.zeros_like(g, dtype=bool)
    mask[rows, idx] = True
    logits = np.where(mask, g, -np.inf).astype(np.float32)
    m = logits.max(axis=1, keepdims=True)
    ex = np.exp(logits - m, dtype=np.float32)
    ex[~mask] = 0.0
    cof = ex / ex.sum(axis=1, keepdims=True)
    return idx, cof.astype(np.float32)


def kernel(x, fc0_w, fc0_b, fc1_w, fc1_b, w1, b1, w2, b2):
    if "pool" not in _cache:
        _cache["pool"] = _build_pool_program()
    if "conv" not in _cache:
        _cache["conv"] = _build_conv_program()

    x = np.ascontiguousarray(x, dtype=np.float32)
    # per-core shards: [SLABS, 128, HWF]
    xs = x.reshape(NCORES, SLABS, 128, HWF)

    # ---- launch 1: pooling ----
    in1 = [{"x": xs[c]} for c in range(NCORES)]
    _last_inmaps["pool"] = in1
    res1 = run_bass_kernel_spmd(_cache["pool"], in1, list(range(NCORES))).results
    pooled_dev = np.stack([res1[c]["pooled"] for c in range(NCORES)])  # [8,2,128,2]
    stats = pooled_dev.reshape(B, C, 2)
    pooled = stats[:, :, 0] + stats[:, :, 1] / float(HWF)  # max + mean, [B, C]

    # ---- host gate ----
    sel, cof = _gate_host(pooled, fc0_w, fc0_b, fc1_w, fc1_b)  # [B,3], [B,E]
    cof3 = cof[np.arange(B)[:, None], sel]  # [B, 3]

    # ---- pack block-diagonal weights ----
    W1 = w1[sel][:, :, :, 0]                      # [B, 3, C, 3, 3]
    W2 = w2[sel][:, :, :, 0] * cof3[:, :, None, None, None]
    # -> [core, slab, j, tap, m=q*64+c]
    def to_taps(Wm):
        Wm = Wm.reshape(NCORES, SLABS, 2, TOPK, C, 3, 3)
        return Wm.transpose(0, 1, 3, 5, 6, 2, 4).reshape(NCORES, SLABS, TOPK, 9, 128)
    W1t, W2t = to_taps(W1), to_taps(W2)
    wd = np.zeros((NCORES, 128, SLABS, TOPK, 9, 128), dtype=np.float32)
    ii = np.arange(128)
    wd1 = wd.copy(); wd2 = wd
    wd1[:, ii, :, :, :, ii] = np.moveaxis(W1t, -1, 0).reshape(128, NCORES, SLABS, TOPK, 9)
    wd2[:, ii, :, :, :, ii] = np.moveaxis(W2t, -1, 0).reshape(128, NCORES, SLABS, TOPK, 9)
    wd1 = wd1.astype(ml_dtypes.bfloat16)
    wd2 = wd2.astype(ml_dtypes.bfloat16)

    B1 = b1[sel]                                   # [B, 3, C]
    B1 = B1.reshape(NCORES, SLABS, 2, TOPK, C).transpose(0, 2, 4, 1, 3)
    B1 = B1.reshape(NCORES, 128, SLABS, TOPK).astype(np.float32)
    BO = np.einsum("bj,bjc->bc", cof3, b2[sel])    # [B, C]
    BO = BO.reshape(NCORES, SLABS, 2, C).transpose(0, 2, 3, 1)
    BO = BO.reshape(NCORES, 128, SLABS).astype(np.float32)

    xb = x.astype(ml_dtypes.bfloat16).reshape(NCORES, SLABS, 128, HWF)

    # ---- launch 2: convs ----
    in2 = [
        {"xb": xb[c], "wd1": wd1[c], "wd2": wd2[c], "b1e": B1[c], "bout": BO[c]}
        for c in range(NCORES)
    ]
    _last_inmaps["conv"] = in2
    res2 = run_bass_kernel_spmd(_cache["conv"], in2, list(range(NCORES))).results
    out = np.stack([res2[c]["out"] for c in range(NCORES)])  # [8, 2, 128, HWF]
    return np.ascontiguousarray(out.reshape(B, C, H, W), dtype=np.float32)


# revision 9
# speedup vs baseline: 1.1252x; 1.1252x over previous
"""MoE depthwise-expert routing kernel for 8 Trainium2 NeuronCores.

Strategy (hardcoded for B=32, C=64, H=W=192, E=6, K=3):
  - Data-parallel over batch: 4 samples/core, processed as 2 slabs of
    (2 samples x 64 channels) = 128 SBUF partitions; spatial dims in the
    free dimension, so conv taps are free-dim offsets and per-channel
    weights are diagonal matmul weights.
  - Launch 1 (device): max+sum pooling over HxW per (sample, channel).
  - Host glue: tiny gate MLP + top-k + softmax (12K FLOPs) selects 3
    experts/sample; packs block-diagonal bf16 weight matrices with the
    softmax coefficient folded into conv2 weights/biases.
  - Launch 2 (device): both depthwise 3x3 convs for the 3 selected
    experts as diagonal matmuls on the TensorE accumulating in PSUM
    (9 taps conv1 -> ACT relu+bias evict; 27 taps conv2 -> ACT bias
    evict) in H-strips with a width-padded layout.
"""

import sys

sys.path.insert(0, "/opt/trn_rl_repo")

import numpy as np
import ml_dtypes

from concourse import bass, bacc, tile
from concourse import mybir
from concourse.bass_utils import run_bass_kernel_spmd

F32 = mybir.dt.float32
BF16 = mybir.dt.bfloat16
AF = mybir.ActivationFunctionType

B, C, H, W = 32, 64, 192, 192
E, TOPK = 6, 3
NCORES = 8
SLABS = 2            # slabs per core, each 2 samples x 64 ch = 128 partitions
HWF = H * W          # 36864 free elems per (sample, channel)
WP = W + 2           # padded row width
R = 32               # output rows per strip
NSTRIP = H // R      # 6
XROWS = R + 4        # x rows per strip (2 halo each side)
LEAD = 2             # flat-buffer lead/trail pad elems
XFLAT = LEAD + XROWS * WP + LEAD
YROWS = R + 4        # y buffer rows: row 0 & 35 scratch, rows 1..34 = y halo+body
YFLAT = LEAD + YROWS * WP + LEAD
C1CHUNKS = [(1, 5), (6, 5), (11, 5), (16, 5), (21, 5), (26, 5), (31, 4)]
TAPS = [(dy, dx) for dy in (-1, 0, 1) for dx in (-1, 0, 1)]

_cache = {}
_last_inmaps = {}


def _build_pool_program():
    nc = bacc.Bacc(None, target_bir_lowering=False, debug=False)
    x = nc.dram_tensor("x", [SLABS, 128, HWF], F32, kind="ExternalInput")
    pooled = nc.dram_tensor("pooled", [SLABS, 128, 2], F32, kind="ExternalOutput")
    CH = 4608
    NCH = HWF // CH  # 8
    with tile.TileContext(nc) as tc:
        with (
            tc.tile_pool(name="xin", bufs=3) as xpool,
            tc.tile_pool(name="acc", bufs=1) as apool,
            tc.tile_pool(name="res", bufs=2) as rpool,
        ):
            mx = apool.tile([128, SLABS * NCH], F32)
            sm = apool.tile([128, SLABS * NCH], F32)
            for s in range(SLABS):
                for i in range(NCH):
                    xt = xpool.tile([128, CH], F32)
                    nc.sync.dma_start(out=xt[:], in_=x[s, :, i * CH:(i + 1) * CH])
                    col = s * NCH + i
                    nc.vector.tensor_reduce(
                        mx[:, col:col + 1], xt[:],
                        mybir.AxisListType.X, mybir.AluOpType.max)
                    nc.vector.tensor_reduce(
                        sm[:, col:col + 1], xt[:],
                        mybir.AxisListType.X, mybir.AluOpType.add)
            for s in range(SLABS):
                pt = rpool.tile([128, 2], F32)
                nc.vector.tensor_reduce(
                    pt[:, 0:1], mx[:, s * NCH:(s + 1) * NCH],
                    mybir.AxisListType.X, mybir.AluOpType.max)
                nc.vector.tensor_reduce(
                    pt[:, 1:2], sm[:, s * NCH:(s + 1) * NCH],
                    mybir.AxisListType.X, mybir.AluOpType.add)
                nc.sync.dma_start(out=pooled[s], in_=pt[:])
    nc.compile()
    return nc


def _build_conv_program():
    nc = bacc.Bacc(None, target_bir_lowering=False, debug=False)
    xb = nc.dram_tensor("xb", [SLABS, 128, HWF], BF16, kind="ExternalInput")
    wd1 = nc.dram_tensor("wd1", [128, SLABS, TOPK, 9, 128], BF16, kind="ExternalInput")
    wd2 = nc.dram_tensor("wd2", [128, SLABS, TOPK, 9, 128], BF16, kind="ExternalInput")
    b1e = nc.dram_tensor("b1e", [128, SLABS, TOPK], F32, kind="ExternalInput")
    bout = nc.dram_tensor("bout", [128, SLABS], F32, kind="ExternalInput")
    out = nc.dram_tensor("out", [SLABS, 128, HWF], F32, kind="ExternalOutput")

    with tile.TileContext(nc) as tc:
        with (
            tc.tile_pool(name="wts", bufs=1) as wpool,
            tc.tile_pool(name="xs", bufs=2) as xpool,
            tc.tile_pool(name="ys", bufs=2) as ypool,
            tc.tile_pool(name="os", bufs=2) as opool,
            tc.tile_pool(name="ps", bufs=4, space="PSUM") as ppool,
        ):
            w1sb = wpool.tile([128, SLABS, TOPK, 9, 128], BF16)
            w2sb = wpool.tile([128, SLABS, TOPK, 9, 128], BF16)
            b1sb = wpool.tile([128, SLABS, TOPK], F32)
            bosb = wpool.tile([128, SLABS], F32)
            nc.sync.dma_start(out=w1sb[:], in_=wd1[:])
            nc.sync.dma_start(out=w2sb[:], in_=wd2[:])
            nc.sync.dma_start(out=b1sb[:], in_=b1e[:])
            nc.sync.dma_start(out=bosb[:], in_=bout[:])

            for s in range(SLABS):
                for st in range(NSTRIP):
                    h0 = st * R
                    xt = xpool.tile([128, XFLAT], BF16, tag="x")
                    x3 = xt[:, LEAD:LEAD + XROWS * WP].rearrange(
                        "p (r c) -> p r c", c=WP)
                    # zero pad columns (image col -1 and 192)
                    nc.vector.memset(x3[:, :, 0:1], 0.0)
                    nc.vector.memset(x3[:, :, W + 1:W + 2], 0.0)
                    # load x rows [h0-2, h0+R+2) ; memset out-of-image rows
                    if st == 0:
                        nc.vector.memset(x3[:, 0:2, 1:W + 1], 0.0)
                        src = xb[s, :, 0:(R + 2) * W].rearrange(
                            "p (r c) -> p r c", c=W)
                        nc.sync.dma_start(out=x3[:, 2:XROWS, 1:W + 1], in_=src)
                    elif st == NSTRIP - 1:
                        nc.vector.memset(x3[:, XROWS - 2:XROWS, 1:W + 1], 0.0)
                        src = xb[s, :, (h0 - 2) * W:H * W].rearrange(
                            "p (r c) -> p r c", c=W)
                        nc.sync.dma_start(out=x3[:, 0:XROWS - 2, 1:W + 1], in_=src)
                    else:
                        src = xb[s, :, (h0 - 2) * W:(h0 + R + 2) * W].rearrange(
                            "p (r c) -> p r c", c=W)
                        nc.sync.dma_start(out=x3[:, :, 1:W + 1], in_=src)

                    # conv1 + relu for the 3 selected experts
                    yts = []
                    for j in range(TOPK):
                        yt = ypool.tile([128, YFLAT], BF16, tag=f"y{j}")
                        y3 = yt[:, LEAD:LEAD + YROWS * WP].rearrange(
                            "p (r c) -> p r c", c=WP)
                        nc.vector.memset(y3[:, :, 0:1], 0.0)
                        nc.vector.memset(y3[:, :, W + 1:W + 2], 0.0)
                        if st == 0:
                            nc.vector.memset(y3[:, 1:2, 1:W + 1], 0.0)
                        if st == NSTRIP - 1:
                            nc.vector.memset(y3[:, YROWS - 2:YROWS - 1, 1:W + 1], 0.0)
                        yts.append(yt)
                        # y rows 1..34 in 2-row PSUM chunks (N=388 <= 512)
                        for cy in range(17):
                            ps = ppool.tile([128, 2, WP], F32, tag="ps")
                            for t, (dy, dx) in enumerate(TAPS):
                                off = LEAD + (1 + 2 * cy + dy) * WP + dx
                                nc.tensor.matmul(
                                    ps[:], w1sb[:, s, j, t, :],
                                    xt[:, off:off + 2 * WP],
                                    start=(t == 0), stop=(t == 8))
                            # evict with relu + bias; skip out-of-image rows
                            r_lo, p_lo = 1 + 2 * cy, 0
                            r_hi = r_lo + 2
                            if st == 0 and cy == 0:
                                r_lo, p_lo = 2, 1
                            if st == NSTRIP - 1 and cy == 16:
                                r_hi -= 1
                            nr = r_hi - r_lo
                            nc.scalar.activation(
                                y3[:, r_lo:r_hi, 1:W + 1],
                                ps[:, p_lo:p_lo + nr, 1:W + 1],
                                AF.Relu, bias=b1sb[:, s, j:j + 1], scale=1.0)

                    # conv2 over 3 experts, combined into one PSUM accumulation
                    ot = opool.tile([128, R, W], F32, tag="o")
                    for co in range(R // 2):
                        ps = ppool.tile([128, 2, WP], F32, tag="ps2")
                        k = 0
                        for j in range(TOPK):
                            for t, (dy, dx) in enumerate(TAPS):
                                off = LEAD + (2 * co + 2 + dy) * WP + dx
                                nc.tensor.matmul(
                                    ps[:], w2sb[:, s, j, t, :],
                                    yts[j][:, off:off + 2 * WP],
                                    start=(k == 0), stop=(k == 26))
                                k += 1
                        nc.scalar.activation(
                            ot[:, 2 * co:2 * co + 2, :],
                            ps[:, :, 1:W + 1],
                            AF.Identity, bias=bosb[:, s:s + 1], scale=1.0)
                    nc.sync.dma_start(
                        out=out[s, :, h0 * W:(h0 + R) * W],
                        in_=ot[:].rearrange("p r c -> p (r c)"))
    nc.compile()
    return nc


def _gate_host(pooled, fc0_w, fc0_b, fc1_w, fc1_b):
    """Replicates reference._gate from pooled [B, C] stats; float32."""
    pooled = pooled.astype(np.float32)
    g_lin = pooled @ fc1_w.T + fc1_b
    g = np.where(g_lin > 0, g_lin, 0.2 * g_lin).astype(np.float32)
    n_lin = (pooled @ fc0_w.T + fc0_b).astype(np.float32)
    noise = (np.log1p(np.exp(-np.abs(n_lin))) + np.maximum(n_lin, 0.0)).astype(np.float32)
    mu = noise.mean(axis=1, keepdims=True)
    sd = noise.std(axis=1, ddof=1, keepdims=True)
    nz = (noise - mu) / sd
    scores = g + nz
    idx = np.argsort(-scores, axis=1, kind="stable")[:, :TOPK]
    rows = np.arange(scores.shape[0])[:, None]
    mask = np.zeros_like(g, dtype=bool)
    mask[rows, idx] = True
    logits = np.where(mask, g, -np.inf).astype(np.float32)
    m = logits.max(axis=1, keepdims=True)
    ex = np.exp(logits - m, dtype=np.float32)
    ex[~mask] = 0.0
    cof = ex / ex.sum(axis=1, keepdims=True)
    return idx, cof.astype(np.float32)


def kernel(x, fc0_w, fc0_b, fc1_w, fc1_b, w1, b1, w2, b2):
    if "pool" not in _cache:
        _cache["pool"] = _build_pool_program()
    if "conv" not in _cache:
        _cache["conv"] = _build_conv_program()

    x = np.ascontiguousarray(x, dtype=np.float32)
    # per-core shards: [SLABS, 128, HWF]
    xs = x.reshape(NCORES, SLABS, 128, HWF)

    # ---- launch 1: pooling ----
    in1 = [{"x": xs[c]} for c in range(NCORES)]
    _last_inmaps["pool"] = in1
    res1 = run_bass_kernel_spmd(_cache["pool"], in1, list(range(NCORES))).results
    pooled_dev = np.stack([res1[c]["pooled"] for c in range(NCORES)])  # [8,2,128,2]
    stats = pooled_dev.reshape(B, C, 2)
    pooled = stats[:, :, 0] + stats[:, :, 1] / float(HWF)  # max + mean, [B, C]

    # ---- host gate ----
    sel, cof = _gate_host(pooled, fc0_w, fc0_b, fc1_w, fc1_b)  # [B,3], [B,E]
    cof3 = cof[np.arange(B)[:, None], sel]  # [B, 3]

    # ---- pack block-diagonal weights ----
    W1 = w1[sel][:, :, :, 0]                      # [B, 3, C, 3, 3]
    W2 = w2[sel][:, :, :, 0] * cof3[:, :, None, None, None]
    # -> [core, slab, j, tap, m=q*64+c]
    def to_taps(Wm):
        Wm = Wm.reshape(NCORES, SLABS, 2, TOPK, C, 3, 3)
        return Wm.transpose(0, 1, 3, 5, 6, 2, 4).reshape(NCORES, SLABS, TOPK, 9, 128)
    W1t, W2t = to_taps(W1), to_taps(W2)
    wd = np.zeros((NCORES, 128, SLABS, TOPK, 9, 128), dtype=np.float32)
    ii = np.arange(128)
    wd1 = wd.copy(); wd2 = wd
    wd1[:, ii, :, :, :, ii] = np.moveaxis(W1t, -1, 0).reshape(128, NCORES, SLABS, TOPK, 9)
    wd2[:, ii, :, :, :, ii] = np.moveaxis(W2t, -1, 0).reshape(128, NCORES, SLABS, TOPK, 9)
    wd1 = wd1.astype(ml_dtypes.bfloat16)
    wd2 = wd2.astype(ml_dtypes.bfloat16)

    B1 = b1[sel]                                   # [B, 3, C]
    B1 = B1.reshape(NCORES, SLABS, 2, TOPK, C).transpose(0, 2, 4, 1, 3)
    B1 = B1.reshape(NCORES, 128, SLABS, TOPK).astype(np.float32)
    BO = np.einsum("bj,bjc->bc", cof3, b2[sel])    # [B, C]
    BO = BO.reshape(NCORES, SLABS, 2, C).transpose(0, 2, 3, 1)
    BO = BO.reshape(NCORES, 128, SLABS).astype(np.float32)

    xb = x.astype(ml_dtypes.bfloat16).reshape(NCORES, SLABS, 128, HWF)

    # ---- launch 2: convs ----
    in2 = [
        {"xb": xb[c], "wd1": wd1[c], "wd2": wd2[c], "b1e": B1[c], "bout": BO[c]}
        for c in range(NCORES)
    ]
    _last_inmaps["conv"] = in2
    res2 = run_bass_kernel_spmd(_cache["conv"], in2, list(range(NCORES))).results
    out = np.stack([res2[c]["out"] for c in range(NCORES)])  # [8, 2, 128, HWF]
    return np.ascontiguousarray(out.reshape(B, C, H, W), dtype=np.float32)


# revision 13
# speedup vs baseline: 3.0477x; 2.7086x over previous
"""MoE depthwise-expert routing kernel for 8 Trainium2 NeuronCores.

Strategy (hardcoded for B=32, C=64, H=W=192, E=6, K=3):
  - Data-parallel over batch: 4 samples/core, processed as 2 slabs of
    (2 samples x 64 channels) = 128 SBUF partitions; spatial dims in the
    free dimension, so conv taps are free-dim offsets and per-channel
    weights are diagonal matmul weights.
  - Launch 1 (device): max+sum pooling over HxW per (sample, channel).
  - Host glue: tiny gate MLP + top-k + softmax (12K FLOPs) selects 3
    experts/sample; packs block-diagonal bf16 weight matrices with the
    softmax coefficient folded into conv2 weights/biases.
  - Launch 2 (device): both depthwise 3x3 convs for the 3 selected
    experts as diagonal matmuls on the TensorE accumulating in PSUM
    (9 taps conv1 -> ACT relu+bias evict; 27 taps conv2 -> ACT bias
    evict) in H-strips with a width-padded layout.
"""

import sys

sys.path.insert(0, "/opt/trn_rl_repo")

import numpy as np
import ml_dtypes

from concourse import bass, bacc, tile
from concourse import mybir
from concourse.bass_utils import run_bass_kernel_spmd

F32 = mybir.dt.float32
BF16 = mybir.dt.bfloat16
AF = mybir.ActivationFunctionType

B, C, H, W = 32, 64, 192, 192
E, TOPK = 6, 3
NCORES = 8
SLABS = 2            # slabs per core, each 2 samples x 64 ch = 128 partitions
HWF = H * W          # 36864 free elems per (sample, channel)
WP = W + 2           # padded row width
R = 32               # output rows per strip
NSTRIP = H // R      # 6
XROWS = R + 4        # x rows per strip (2 halo each side)
LEAD = 2             # flat-buffer lead/trail pad elems
XFLAT = LEAD + XROWS * WP + LEAD
YROWS = R + 4        # y buffer rows: row 0 & 35 scratch, rows 1..34 = y halo+body
YFLAT = LEAD + YROWS * WP + LEAD
C1CHUNKS = [(1, 5), (6, 5), (11, 5), (16, 5), (21, 5), (26, 5), (31, 4)]
TAPS = [(dy, dx) for dy in (-1, 0, 1) for dx in (-1, 0, 1)]

_cache = {}
_last_inmaps = {}


def _build_pool_program():
    nc = bacc.Bacc(None, target_bir_lowering=False, debug=False)
    x = nc.dram_tensor("xb", [SLABS, 128, HWF], BF16, kind="ExternalInput")
    pooled = nc.dram_tensor("pooled", [SLABS, 128, 2], F32, kind="ExternalOutput")
    CH = 4608
    NCH = HWF // CH  # 8
    with tile.TileContext(nc) as tc:
        with (
            tc.tile_pool(name="xin", bufs=3) as xpool,
            tc.tile_pool(name="scr", bufs=2) as spool,
            tc.tile_pool(name="acc", bufs=1) as apool,
            tc.tile_pool(name="res", bufs=2) as rpool,
        ):
            mx = apool.tile([128, SLABS * NCH], F32)
            sm = apool.tile([128, SLABS * NCH], F32)
            for s in range(SLABS):
                for i in range(NCH):
                    xt = xpool.tile([128, CH], BF16)
                    nc.sync.dma_start(out=xt[:], in_=x[s, :, i * CH:(i + 1) * CH])
                    col = s * NCH + i
                    # max on DVE; sum on ACT via accum_out (engines split)
                    nc.vector.tensor_reduce(
                        mx[:, col:col + 1], xt[:],
                        mybir.AxisListType.X, mybir.AluOpType.max)
                    sc = spool.tile([128, CH], BF16, tag="sc")
                    nc.scalar.activation(
                        sc[:], xt[:], AF.Copy, bias=0.0, scale=1.0,
                        accum_out=sm[:, col:col + 1])
            for s in range(SLABS):
                pt = rpool.tile([128, 2], F32)
                nc.vector.tensor_reduce(
                    pt[:, 0:1], mx[:, s * NCH:(s + 1) * NCH],
                    mybir.AxisListType.X, mybir.AluOpType.max)
                nc.vector.tensor_reduce(
                    pt[:, 1:2], sm[:, s * NCH:(s + 1) * NCH],
                    mybir.AxisListType.X, mybir.AluOpType.add)
                nc.sync.dma_start(out=pooled[s], in_=pt[:])
    nc.compile()
    return nc


def _build_conv_program():
    nc = bacc.Bacc(None, target_bir_lowering=False, debug=False)
    xb = nc.dram_tensor("xb", [SLABS, 128, HWF], BF16, kind="ExternalInput")
    wd1 = nc.dram_tensor("wd1", [128, SLABS, TOPK, 9, 128], BF16, kind="ExternalInput")
    wd2 = nc.dram_tensor("wd2", [128, SLABS, TOPK, 9, 128], BF16, kind="ExternalInput")
    b1e = nc.dram_tensor("b1e", [128, SLABS, TOPK], F32, kind="ExternalInput")
    bout = nc.dram_tensor("bout", [128, SLABS], F32, kind="ExternalInput")
    out = nc.dram_tensor("out", [SLABS, 128, HWF], F32, kind="ExternalOutput")

    with tile.TileContext(nc) as tc:
        with (
            tc.tile_pool(name="wts", bufs=1) as wpool,
            tc.tile_pool(name="xs", bufs=2) as xpool,
            tc.tile_pool(name="ys", bufs=2) as ypool,
            tc.tile_pool(name="os", bufs=2) as opool,
            tc.tile_pool(name="ps", bufs=4, space="PSUM") as ppool,
        ):
            w1sb = wpool.tile([128, SLABS, TOPK, 9, 128], BF16)
            w2sb = wpool.tile([128, SLABS, TOPK, 9, 128], BF16)
            b1sb = wpool.tile([128, SLABS, TOPK], F32)
            bosb = wpool.tile([128, SLABS], F32)
            nc.sync.dma_start(out=w1sb[:], in_=wd1[:])
            nc.sync.dma_start(out=w2sb[:], in_=wd2[:])
            nc.sync.dma_start(out=b1sb[:], in_=b1e[:])
            nc.sync.dma_start(out=bosb[:], in_=bout[:])

            for s in range(SLABS):
                for st in range(NSTRIP):
                    h0 = st * R
                    xt = xpool.tile([128, XFLAT], BF16, tag="x")
                    x3 = xt[:, LEAD:LEAD + XROWS * WP].rearrange(
                        "p (r c) -> p r c", c=WP)
                    # zero pad columns (image col -1 and 192)
                    nc.vector.memset(x3[:, :, 0:1], 0.0)
                    nc.vector.memset(x3[:, :, W + 1:W + 2], 0.0)
                    # load x rows [h0-2, h0+R+2) ; memset out-of-image rows
                    if st == 0:
                        nc.vector.memset(x3[:, 0:2, 1:W + 1], 0.0)
                        src = xb[s, :, 0:(R + 2) * W].rearrange(
                            "p (r c) -> p r c", c=W)
                        nc.sync.dma_start(out=x3[:, 2:XROWS, 1:W + 1], in_=src)
                    elif st == NSTRIP - 1:
                        nc.vector.memset(x3[:, XROWS - 2:XROWS, 1:W + 1], 0.0)
                        src = xb[s, :, (h0 - 2) * W:H * W].rearrange(
                            "p (r c) -> p r c", c=W)
                        nc.sync.dma_start(out=x3[:, 0:XROWS - 2, 1:W + 1], in_=src)
                    else:
                        src = xb[s, :, (h0 - 2) * W:(h0 + R + 2) * W].rearrange(
                            "p (r c) -> p r c", c=W)
                        nc.sync.dma_start(out=x3[:, :, 1:W + 1], in_=src)

                    # conv1 + relu for the 3 selected experts.
                    # y rows 1..34 computed as flat 512-col PSUM chunks; pad
                    # columns get junk and are re-zeroed after eviction.
                    yts = []
                    y_lo, y_hi = LEAD + WP, LEAD + (YROWS - 1) * WP
                    for j in range(TOPK):
                        yt = ypool.tile([128, YFLAT], BF16, tag=f"y{j}")
                        y3 = yt[:, LEAD:LEAD + YROWS * WP].rearrange(
                            "p (r c) -> p r c", c=WP)
                        yts.append(yt)
                        for f0 in range(y_lo, y_hi, 512):
                            n = min(512, y_hi - f0)
                            ps = ppool.tile([128, 512], F32, tag="ps")
                            for t, (dy, dx) in enumerate(TAPS):
                                off = f0 + dy * WP + dx
                                nc.tensor.matmul(
                                    ps[:, 0:n], w1sb[:, s, j, t, :],
                                    xt[:, off:off + n],
                                    start=(t == 0), stop=(t == 8))
                            nc.scalar.activation(
                                yt[:, f0:f0 + n], ps[:, 0:n],
                                AF.Relu, bias=b1sb[:, s, j:j + 1], scale=1.0)
                        # restore zero pad columns clobbered by flat evicts
                        nc.vector.memset(y3[:, :, 0:1], 0.0)
                        nc.vector.memset(y3[:, :, W + 1:W + 2], 0.0)
                        if st == 0:
                            nc.vector.memset(y3[:, 1:2, 1:W + 1], 0.0)
                        if st == NSTRIP - 1:
                            nc.vector.memset(y3[:, YROWS - 2:YROWS - 1, 1:W + 1], 0.0)

                    # conv2 over 3 experts, combined into one PSUM
                    # accumulation; padded out buffer, flat 512-col chunks.
                    ot = opool.tile([128, R * WP], F32, tag="o")
                    for f0 in range(0, R * WP, 512):
                        n = min(512, R * WP - f0)
                        ps = ppool.tile([128, 512], F32, tag="ps2")
                        k = 0
                        for j in range(TOPK):
                            for t, (dy, dx) in enumerate(TAPS):
                                off = f0 + LEAD + (2 + dy) * WP + dx
                                nc.tensor.matmul(
                                    ps[:, 0:n], w2sb[:, s, j, t, :],
                                    yts[j][:, off:off + n],
                                    start=(k == 0), stop=(k == 26))
                                k += 1
                        nc.scalar.activation(
                            ot[:, f0:f0 + n], ps[:, 0:n],
                            AF.Identity, bias=bosb[:, s:s + 1], scale=1.0)
                    nc.sync.dma_start(
                        out=out[s, :, h0 * W:(h0 + R) * W].rearrange(
                            "p (r c) -> p r c", c=W),
                        in_=ot[:].rearrange("p (r c) -> p r c", c=WP)[:, :, 1:W + 1])
    nc.compile()
    return nc


def _gate_host(pooled, fc0_w, fc0_b, fc1_w, fc1_b):
    """Replicates reference._gate from pooled [B, C] stats; float32."""
    pooled = pooled.astype(np.float32)
    g_lin = pooled @ fc1_w.T + fc1_b
    g = np.where(g_lin > 0, g_lin, 0.2 * g_lin).astype(np.float32)
    n_lin = (pooled @ fc0_w.T + fc0_b).astype(np.float32)
    noise = (np.log1p(np.exp(-np.abs(n_lin))) + np.maximum(n_lin, 0.0)).astype(np.float32)
    mu = noise.mean(axis=1, keepdims=True)
    sd = noise.std(axis=1, ddof=1, keepdims=True)
    nz = (noise - mu) / sd
    scores = g + nz
    idx = np.argsort(-scores, axis=1, kind="stable")[:, :TOPK]
    rows = np.arange(scores.shape[0])[:, None]
    mask = np.zeros_like(g, dtype=bool)
    mask[rows, idx] = True
    logits = np.where(mask, g, -np.inf).astype(np.float32)
    m = logits.max(axis=1, keepdims=True)
    ex = np.exp(logits - m, dtype=np.float32)
    ex[~mask] = 0.0
    cof = ex / ex.sum(axis=1, keepdims=True)
    return idx, cof.astype(np.float32)


def kernel(x, fc0_w, fc0_b, fc1_w, fc1_b, w1, b1, w2, b2):
    if "pool" not in _cache:
        _cache["pool"] = _build_pool_program()
    if "conv" not in _cache:
        _cache["conv"] = _build_conv_program()

    x = np.ascontiguousarray(x, dtype=np.float32)
    xb = x.astype(ml_dtypes.bfloat16).reshape(NCORES, SLABS, 128, HWF)

    # ---- launch 1: pooling (bf16 x; selection margin verified safe) ----
    in1 = [{"xb": xb[c]} for c in range(NCORES)]
    _last_inmaps["pool"] = in1
    res1 = run_bass_kernel_spmd(_cache["pool"], in1, list(range(NCORES))).results
    pooled_dev = np.stack([res1[c]["pooled"] for c in range(NCORES)])  # [8,2,128,2]
    stats = pooled_dev.reshape(B, C, 2)
    pooled = stats[:, :, 0] + stats[:, :, 1] / float(HWF)  # max + mean, [B, C]

    # ---- host gate ----
    sel, cof = _gate_host(pooled, fc0_w, fc0_b, fc1_w, fc1_b)  # [B,3], [B,E]
    cof3 = cof[np.arange(B)[:, None], sel]  # [B, 3]

    # ---- pack block-diagonal weights ----
    W1 = w1[sel][:, :, :, 0]                      # [B, 3, C, 3, 3]
    W2 = w2[sel][:, :, :, 0] * cof3[:, :, None, None, None]
    # -> [core, slab, j, tap, m=q*64+c]
    def to_taps(Wm):
        Wm = Wm.reshape(NCORES, SLABS, 2, TOPK, C, 3, 3)
        return Wm.transpose(0, 1, 3, 5, 6, 2, 4).reshape(NCORES, SLABS, TOPK, 9, 128)
    W1t, W2t = to_taps(W1), to_taps(W2)
    wd = np.zeros((NCORES, 128, SLABS, TOPK, 9, 128), dtype=np.float32)
    ii = np.arange(128)
    wd1 = wd.copy(); wd2 = wd
    wd1[:, ii, :, :, :, ii] = np.moveaxis(W1t, -1, 0).reshape(128, NCORES, SLABS, TOPK, 9)
    wd2[:, ii, :, :, :, ii] = np.moveaxis(W2t, -1, 0).reshape(128, NCORES, SLABS, TOPK, 9)
    wd1 = wd1.astype(ml_dtypes.bfloat16)
    wd2 = wd2.astype(ml_dtypes.bfloat16)

    B1 = b1[sel]                                   # [B, 3, C]
    B1 = B1.reshape(NCORES, SLABS, 2, TOPK, C).transpose(0, 2, 4, 1, 3)
    B1 = B1.reshape(NCORES, 128, SLABS, TOPK).astype(np.float32)
    BO = np.einsum("bj,bjc->bc", cof3, b2[sel])    # [B, C]
    BO = BO.reshape(NCORES, SLABS, 2, C).transpose(0, 2, 3, 1)
    BO = BO.reshape(NCORES, 128, SLABS).astype(np.float32)

    # ---- launch 2: convs ----
    in2 = [
        {"xb": xb[c], "wd1": wd1[c], "wd2": wd2[c], "b1e": B1[c], "bout": BO[c]}
        for c in range(NCORES)
    ]
    _last_inmaps["conv"] = in2
    res2 = run_bass_kernel_spmd(_cache["conv"], in2, list(range(NCORES))).results
    out = np.stack([res2[c]["out"] for c in range(NCORES)])  # [8, 2, 128, HWF]
    return np.ascontiguousarray(out.reshape(B, C, H, W), dtype=np.float32)
